# revision 1
# baseline (speedup 1.0000x reference)
"""Trainium2 Bass kernel for nn_Graph_CNN_ortega (3-branch spectral GCN, 3 layers).

Strategy (data-parallel over batch, 8 items per core, no collectives, fp32-exact):
  Layer-synchronous phases per (layer l, branch k); U and U^T are streamed
  from HBM as [128,512] slabs, each slab reused by all 8 items' matmuls,
  so U traffic is 24MB/layer/core independent of batch:

    A-phase: agg^T[b] = sum_jc h[b][jc].T @ U[k][jc, :]
             (lhsT = h tile, rhs = U slab, psum [D,512] per item, 8 banks)
    B/C per item:
             t^T  = relu(w1[k].T @ agg^T + b1)
             m[jc]= (t^T[:, jc]).T @ w2_eff[k] (+b2 on evac)   (natural layout)
    D-phase: out^T[b] += sum_jc m[b][jc].T' : lhsT = m tile, rhs = U^T slab
             accumulated over jc in PSUM, over branches k in SBUF (o_acc).
             softmax(bw) folded into w2/b2 on host.
    finalize: h_next = relu(out^T).T via PE transposes (layers 0,1);
              layer 2: pooled[:, b] = rowsum(relu(out^T)) (mean -> Wc1).
  Classifier: z^T = Wc1.T @ pooled ; PReLU ; logits^T = Wc2.T @ z.
"""

import sys

for _p in ("/opt/trn_rl_repo", "/root/.axon_site/_ro/trn_rl_repo"):
    if _p not in sys.path:
        sys.path.append(_p)

import numpy as np

N_CORES = 8
B, N, DIN, DH, H, L, C = 64, 1024, 64, 128, 128, 3, 4
BL = B // N_CORES  # items per core
NJ = N // 128      # 8 j-chunks
NI = N // 512      # 2 i-chunks of 512

_CACHE = {}


def _build_program():
    import concourse.bass as bass  # noqa: F401
    from concourse import bacc, mybir
    import concourse.tile as tile

    f32 = mybir.dt.float32
    f32r = mybir.dt.float32r
    AF = mybir.ActivationFunctionType

    nc = bacc.Bacc("TRN2", target_bir_lowering=False, debug=False,
                   num_devices=N_CORES)

    # ---- DRAM parameters (host pre-tiled layouts) ----
    x_d = nc.dram_tensor("x", [BL, NJ, 128, DIN], f32r, kind="ExternalInput")
    u_d = nc.dram_tensor("u", [3, NJ, 128, N], f32r, kind="ExternalInput")
    ut_d = nc.dram_tensor("ut", [3, NJ, 128, N], f32r, kind="ExternalInput")
    w1a_d = nc.dram_tensor("w1a", [DIN, 3, H], f32r, kind="ExternalInput")
    w1b_d = nc.dram_tensor("w1b", [DH, L - 1, 3, H], f32r, kind="ExternalInput")
    w2_d = nc.dram_tensor("w2", [H, L, 3, DH], f32, kind="ExternalInput")
    b1_d = nc.dram_tensor("b1", [H, L, 3], f32, kind="ExternalInput")
    b2_d = nc.dram_tensor("b2", [128, L, 3, DH], f32, kind="ExternalInput")
    wc1_d = nc.dram_tensor("wc1", [DH, 128], f32, kind="ExternalInput")
    bc1_d = nc.dram_tensor("bc1", [128, 1], f32, kind="ExternalInput")
    al_d = nc.dram_tensor("alpha", [128, 1], f32, kind="ExternalInput")
    wc2_d = nc.dram_tensor("wc2", [128, C], f32, kind="ExternalInput")
    bc2_d = nc.dram_tensor("bc2", [C, 1], f32, kind="ExternalInput")
    id_d = nc.dram_tensor("ident", [128, 128], f32, kind="ExternalInput")
    y_d = nc.dram_tensor("y", [C, BL], f32, kind="ExternalOutput")

    from contextlib import ExitStack

    with tile.TileContext(nc) as tc, ExitStack() as ctx:
        const = ctx.enter_context(tc.tile_pool(name="const", bufs=1))
        slabs = ctx.enter_context(tc.tile_pool(name="slabs", bufs=6))
        aggp = ctx.enter_context(tc.tile_pool(name="aggp", bufs=BL))
        tp = ctx.enter_context(tc.tile_pool(name="tp", bufs=2))
        mp = ctx.enter_context(tc.tile_pool(name="mp", bufs=BL))
        op = ctx.enter_context(tc.tile_pool(name="op", bufs=BL))
        hp = ctx.enter_context(tc.tile_pool(name="hp", bufs=BL))
        ps = ctx.enter_context(tc.tile_pool(name="ps", bufs=8, space="PSUM"))

        # ---- resident small tensors ----
        x_sb = const.tile([128, BL, NJ, DIN], f32r, tag="x")
        for b in range(BL):
            for jc in range(NJ):
                nc.sync.dma_start(out=x_sb[:, b, jc, :], in_=x_d.ap()[b, jc])

        w1a_sb = const.tile([DIN, 3, H], f32r, tag="w1a")
        nc.sync.dma_start(out=w1a_sb[:], in_=w1a_d.ap())
        w1b_sb = const.tile([DH, L - 1, 3, H], f32r, tag="w1b")
        nc.sync.dma_start(out=w1b_sb[:], in_=w1b_d.ap())
        w2_sb = const.tile([H, L, 3, DH], f32, tag="w2")
        nc.sync.dma_start(out=w2_sb[:], in_=w2_d.ap())
        b1_sb = const.tile([H, L, 3], f32, tag="b1")
        nc.sync.dma_start(out=b1_sb[:], in_=b1_d.ap())
        b2_sb = const.tile([128, L, 3, DH], f32, tag="b2")
        nc.sync.dma_start(out=b2_sb[:], in_=b2_d.ap())
        wc1_sb = const.tile([DH, 128], f32, tag="wc1")
        nc.sync.dma_start(out=wc1_sb[:], in_=wc1_d.ap())
        bc1_sb = const.tile([128, 1], f32, tag="bc1")
        nc.sync.dma_start(out=bc1_sb[:], in_=bc1_d.ap())
        al_sb = const.tile([128, 1], f32, tag="al")
        nc.sync.dma_start(out=al_sb[:], in_=al_d.ap())
        wc2_sb = const.tile([128, C], f32, tag="wc2")
        nc.sync.dma_start(out=wc2_sb[:], in_=wc2_d.ap())
        bc2_sb = const.tile([C, 1], f32, tag="bc2")
        nc.sync.dma_start(out=bc2_sb[:], in_=bc2_d.ap())
        id_sb = const.tile([128, 128], f32, tag="id")
        nc.sync.dma_start(out=id_sb[:], in_=id_d.ap())

        pooled = const.tile([DH, BL], f32, tag="pooled")

        mm = nc.tensor.matmul
        h_cur = [None] * BL  # SBUF [128, NJ, DH] per item for l > 0

        for l in range(L):
            D = DIN if l == 0 else DH

            def lhs_h(b, jc):
                if l == 0:
                    return x_sb[:, b, jc, :]
                return h_cur[b][:, jc, :]

            o_accs = [None] * BL
            for k in range(3):
                # ---- A phase: agg^T for all items, U[k] streamed ----
                agg_sbs = [aggp.tile([D, N], f32r, tag="aggsb", name="aggsb")
                           for _ in range(BL)]
                for ic in range(NI):
                    ps_a = [ps.tile([D, 512], f32, tag="ps", name="psa")
                            for _ in range(BL)]
                    for jc in range(NJ):
                        slab = slabs.tile([128, 512], f32r, tag="uslab")
                        nc.sync.dma_start(
                            out=slab[:],
                            in_=u_d.ap()[k, jc][:, ic * 512:(ic + 1) * 512])
                        for b in range(BL):
                            mm(ps_a[b][:], lhsT=lhs_h(b, jc), rhs=slab[:],
                               start=(jc == 0), stop=(jc == NJ - 1))
                    for b in range(BL):
                        nc.vector.tensor_copy(
                            out=agg_sbs[b][:, ic * 512:(ic + 1) * 512],
                            in_=ps_a[b][:])

                # ---- B/C per item ----
                m_sts = []
                w1s = w1a_sb[:, k, :] if l == 0 else w1b_sb[:, l - 1, k, :]
                for b in range(BL):
                    t_sb = tp.tile([H, N], f32, tag="tsb")
                    for ic in range(NI):
                        ps_t = ps.tile([H, 512], f32, tag="ps")
                        mm(ps_t[:], lhsT=w1s,
                           rhs=agg_sbs[b][:, ic * 512:(ic + 1) * 512],
                           start=True, stop=True)
                        nc.scalar.activation(
                            out=t_sb[:, ic * 512:(ic + 1) * 512], in_=ps_t[:],
                            func=AF.Relu, bias=b1_sb[:, l, k:k + 1], scale=1.0)
                    m_st = mp.tile([128, NJ, DH], f32r, tag="mst")
                    for half in range(2):
                        ps_m = ps.tile([128, 512], f32, tag="ps")
                        for q in range(4):
                            jc = half * 4 + q
                            mm(ps_m[:, q * 128:(q + 1) * 128],
                               lhsT=t_sb[:, jc * 128:(jc + 1) * 128],
                               rhs=w2_sb[:, l, k, :], start=True, stop=True)
                        for q in range(4):
                            jc = half * 4 + q
                            nc.vector.tensor_add(
                                out=m_st[:, jc, :],
                                in0=ps_m[:, q * 128:(q + 1) * 128],
                                in1=b2_sb[:, l, k, :])
                    m_sts.append(m_st)

                # ---- D phase: out^T += m.T' x U^T[k], slabs streamed ----
                if k == 0:
                    for b in range(BL):
                        o_accs[b] = op.tile([DH, N], f32, tag="oacc", name="oacc")
                for ic in range(NI):
                    ps_o = [ps.tile([DH, 512], f32, tag="ps", name="pso")
                            for _ in range(BL)]
                    for jc in range(NJ):
                        slab = slabs.tile([128, 512], f32r, tag="uslab")
                        nc.sync.dma_start(
                            out=slab[:],
                            in_=ut_d.ap()[k, jc][:, ic * 512:(ic + 1) * 512])
                        for b in range(BL):
                            mm(ps_o[b][:], lhsT=m_sts[b][:, jc, :], rhs=slab[:],
                               start=(jc == 0), stop=(jc == NJ - 1))
                    for b in range(BL):
                        dst = o_accs[b][:, ic * 512:(ic + 1) * 512]
                        if k == 0:
                            nc.vector.tensor_copy(out=dst, in_=ps_o[b][:])
                        else:
                            nc.vector.tensor_add(out=dst, in0=dst,
                                                 in1=ps_o[b][:])

            # ---- finalize per item ----
            for b in range(BL):
                if l < L - 1:
                    hn = hp.tile([128, NJ, DH], f32r, tag="h")
                    for half in range(2):
                        ps_tr = ps.tile([128, 512], f32, tag="ps")
                        for q in range(4):
                            jc = half * 4 + q
                            nc.tensor.transpose(
                                ps_tr[:, q * 128:(q + 1) * 128],
                                o_accs[b][:, jc * 128:(jc + 1) * 128],
                                id_sb[:])
                        nc.vector.tensor_scalar_max(
                            out=hn[:, half * 4:(half + 1) * 4, :],
                            in0=ps_tr[:], scalar1=0.0)
                    h_cur[b] = hn
                else:
                    orl = tp.tile([DH, N], f32, tag="tsb")
                    nc.vector.tensor_scalar_max(out=orl[:], in0=o_accs[b][:],
                                                scalar1=0.0)
                    nc.vector.reduce_sum(out=pooled[:, b:b + 1], in_=orl[:],
                                         axis=mybir.AxisListType.X)

        # ---- classifier ----
        ps_z = ps.tile([128, BL], f32, tag="ps")
        mm(ps_z[:], lhsT=wc1_sb[:], rhs=pooled[:], start=True, stop=True)
        pos = tp.tile([128, BL], f32, tag="cls_pos")
        tot = tp.tile([128, BL], f32, tag="cls_tot")
        nc.scalar.activation(out=pos[:], in_=ps_z[:], func=AF.Relu,
                             bias=bc1_sb[:, 0:1], scale=1.0)
        nc.scalar.activation(out=tot[:], in_=ps_z[:], func=AF.Identity,
                             bias=bc1_sb[:, 0:1], scale=1.0)
        nc.vector.tensor_sub(out=tot[:], in0=tot[:], in1=pos[:])
        nc.vector.tensor_scalar_mul(out=tot[:], in0=tot[:],
                                    scalar1=al_sb[:, 0:1])
        nc.vector.tensor_add(out=pos[:], in0=pos[:], in1=tot[:])
        ps_c = ps.tile([C, BL], f32, tag="ps")
        mm(ps_c[:], lhsT=wc2_sb[:], rhs=pos[:], start=True, stop=True)
        y_sb = tp.tile([C, BL], f32, tag="ysb")
        nc.scalar.activation(out=y_sb[:], in_=ps_c[:], func=AF.Identity,
                             bias=bc2_sb[:, 0:1], scale=1.0)
        nc.sync.dma_start(out=y_d.ap(), in_=y_sb[:])

    nc.compile()
    return nc


def _get_program():
    if "nc" not in _CACHE:
        _CACHE["nc"] = _build_program()
    return _CACHE["nc"]


def _prep_inputs(x, U, w1_0, b1_0, w2_0, b2_0, w1_r, b1_r, w2_r, b2_r,
                 bw, Wc1, bc1, alpha, Wc2, bc2):
    """Host-side weight prep shared by all cores. Returns dict of common arrays."""
    f = np.float32
    bw = np.asarray(bw, f)
    e = np.exp(bw - bw.max(axis=1, keepdims=True))
    ws = e / e.sum(axis=1, keepdims=True)          # [L, 3] softmax per layer

    w2_all = np.empty((H, L, 3, DH), f)
    b2_all = np.empty((128, L, 3, DH), f)
    b1_all = np.empty((H, L, 3), f)
    for l in range(L):
        w2_l = np.asarray(w2_0 if l == 0 else w2_r[l - 1], f)  # [3,H,DH]
        b2_l = np.asarray(b2_0 if l == 0 else b2_r[l - 1], f)  # [3,DH]
        b1_l = np.asarray(b1_0 if l == 0 else b1_r[l - 1], f)  # [3,H]
        for k in range(3):
            w2_all[:, l, k, :] = w2_l[k] * ws[l, k]
            b2_all[:, l, k, :] = (b2_l[k] * ws[l, k])[None, :]
            b1_all[:, l, k] = b1_l[k]

    U = np.asarray(U, f)
    return {
        "u": np.ascontiguousarray(U.reshape(3, NJ, 128, N)),
        "ut": np.ascontiguousarray(U.transpose(0, 2, 1).reshape(3, NJ, 128, N)),
        "w1a": np.ascontiguousarray(np.asarray(w1_0, f).transpose(1, 0, 2)),
        "w1b": np.ascontiguousarray(np.asarray(w1_r, f).transpose(2, 0, 1, 3)),
        "w2": w2_all,
        "b1": b1_all,
        "b2": b2_all,
        "wc1": np.asarray(Wc1, f) / np.float32(N),
        "bc1": np.asarray(bc1, f).reshape(128, 1),
        "alpha": np.asarray(alpha, f).reshape(128, 1),
        "wc2": np.asarray(Wc2, f),
        "bc2": np.asarray(bc2, f).reshape(C, 1),
        "ident": np.eye(128, dtype=f),
    }


def kernel(x, U, w1_0, b1_0, w2_0, b2_0, w1_r, b1_r, w2_r, b2_r,
           bw, Wc1, bc1, alpha, Wc2, bc2, _trace=False, _trace_kwargs=None):
    from concourse.bass_utils import run_bass_kernel_spmd

    nc = _get_program()
    common = _prep_inputs(x, U, w1_0, b1_0, w2_0, b2_0, w1_r, b1_r,
                          w2_r, b2_r, bw, Wc1, bc1, alpha, Wc2, bc2)
    x = np.asarray(x, np.float32)
    in_maps = []
    for c in range(N_CORES):
        m = dict(common)
        m["x"] = np.ascontiguousarray(
            x[c * BL:(c + 1) * BL].reshape(BL, NJ, 128, DIN))
        in_maps.append(m)

    kwargs = {}
    if _trace:
        kwargs.update(trace=True, **(_trace_kwargs or {}))
    res = run_bass_kernel_spmd(nc, in_maps, list(range(N_CORES)), **kwargs)
    out = np.concatenate([res.results[c]["y"].T for c in range(N_CORES)], axis=0)
    if _trace:
        return out.astype(np.float32), res
    return out.astype(np.float32)

